# revision 10
# baseline (speedup 1.0000x reference)
"""Trainium2 Bass kernel for topk_masking:  out = X + alpha * (top32_mask(A) @ X).

Row-parallel across 8 NeuronCores (A sharded [1024, 8192] per core, X
replicated).  A is shipped to the device as fp16 of (A - SHIFT) where
SHIFT ~ the expected 32nd-largest value per row: rounding is monotone, so
the fp16 top-32 equals the fp32 top-32 unless two values straddling the
rank-32 boundary collide on the fp16 grid -- precisely the case the
count detector flags for exact host recomputation (the shift centers the
threshold near 0 where the fp16 grid is finest; ~0.5% of rows collide).
This halves the dominant A DMA traffic and doubles VectorE scan rate.

Per 128-row batch on each core (engine assignment chosen so no engine's
strict-FIFO queue ever holds an instruction that waits on a *later*
pipeline stage -- that coupling, not bandwidth, limited earlier versions):
  * DMA (sync ring): one 2 MB load of the fp16 A row-block.
  * VectorE: per-512-segment max8 -> candidate top-8s, then 4 rounds of
    max+match_replace over the candidates -> top-32 values; t32 = 32nd
    largest.  Exact unless >8 of a row's top-32 fall in one segment
    (detected and host-fixed; ~9 rows for this data).
  * ScalarE: maskpm = Sign(A + (2^-25 - t32)) over the full row in fp8e4
    (+1 selected, -1 not; the 2^-25 bump keeps exact-boundary values
    selected), fused accum_out -> per-row count detector (catches segment
    overflow, fp16 boundary ties, Sign==0).
  * GPSIMD dma_gather(transpose): full-row fp8 mask -> transposed tokens
    (SWDGE on Pool).  The xbar transposes at 16-bit granularity, so
    transposed tokens hold column PAIRS; the matmul consumes them with
    stride-2 fp8 weight APs (even/odd) against X pre-split into even/odd
    row copies (host-side layout).
  * TensorE: psum = sum_c maskpm_c @ Xs_c  (64 accumulated matmuls, fp8
    weights x bf16 moving), where Xs = (alpha/2) X (host-scaled).
  * VectorE (deferred one batch so it cannot stall the next batch's scan
    behind the matmul wait): out = psum + xmod, with host-precomputed
    xmod = X_self + (alpha/2) colsum(Xs-consistent); store via sync ring.
Host: rows whose detector count != 2K - N are recomputed exactly.
"""

import os
import numpy as np

N = 8192
D = 256
K = 32
NCORES = 8
RPC = N // NCORES          # rows per core = 1024
BATCH = 128
NBATCH = RPC // BATCH      # 8
SEG = int(os.environ.get("TOPK_SEG", "512"))
NCH2 = N // 256            # 32 token-chunks (column pairs x 128)
NEG_BIG = -60000.0         # fp16-representable sentinel
SHIFT = 2.66               # ~E[32nd largest of 8192 N(0,1)] -- fp16 grid is
                           # finest near 0, so center the threshold there
EPS_TIE = float(2.0 ** -25)  # keeps Sign(y - t32) > 0 for y == t32

last_results = None
_nc_cache = {}


def _build_cached(loop_reps=1, seg=None):
    key = (loop_reps, seg or SEG)
    if key not in _nc_cache:
        _nc_cache[key] = _build(loop_reps, seg)
    return _nc_cache[key]


def _build(loop_reps=1, seg=None):
    import concourse.bacc as bacc
    import concourse.mybir as mybir
    from concourse.tile import TileContext
    from concourse import library_config

    seg = seg or SEG
    nseg = N // seg            # segments per full row
    fp32 = mybir.dt.float32
    fp16 = mybir.dt.float16
    bf16 = mybir.dt.bfloat16
    fp8 = mybir.dt.float8e4
    u16 = mybir.dt.uint16
    add = mybir.AluOpType.add
    mult = mybir.AluOpType.mult
    Sign = mybir.ActivationFunctionType.Sign
    Copy = mybir.ActivationFunctionType.Copy

    nc = bacc.Bacc("TRN2", debug=False)
    a_in = nc.declare_dram_parameter("a", [RPC, N], fp16, isOutput=False)
    xb_in = nc.declare_dram_parameter("xb", [128, NCH2 * 2 * D], bf16, isOutput=False)
    xm_in = nc.declare_dram_parameter("xmod", [RPC, D], fp32, isOutput=False)
    ti_in = nc.declare_dram_parameter("tidx", [128, 8], mybir.dt.int16, isOutput=False)
    out_ext = nc.declare_dram_parameter("out", [RPC, D], fp32, isOutput=True)
    cnt_ext = nc.declare_dram_parameter("count", [RPC, 1], fp32, isOutput=True)

    abufs = int(os.environ.get("TOPK_ABUFS", "4"))

    with TileContext(nc) as tc:
        with (
            tc.tile_pool(name="persist", bufs=1) as persist,
            tc.tile_pool(name="apool", bufs=abufs) as apool,
            tc.tile_pool(name="mpool", bufs=int(os.environ.get("TOPK_MBUFS", "3"))) as mpool,
            tc.tile_pool(name="mtpool", bufs=int(os.environ.get("TOPK_MTBUFS", "3"))) as mtpool,
            tc.tile_pool(name="small", bufs=3) as small,
            tc.tile_pool(name="psum", bufs=2, space="PSUM") as psum_pool,
        ):
            nc.gpsimd.load_library(library_config.mlp)

            tidx = persist.tile([128, 8], mybir.dt.int16)
            nc.scalar.dma_start(out=tidx[:], in_=ti_in[:])

            at_tiles = {}

            def load_at(b):
                atile = apool.tile([128, N], fp16, tag="at")
                nc.sync.dma_start(
                    out=atile[:], in_=a_in[b * BATCH:(b + 1) * BATCH, :])
                at_tiles[b] = atile

            if loop_reps == 1:
                load_at(0)
                load_at(1)

            # Xs resident in bf16, even/odd token-chunk layout, pre-scaled by
            # alpha/2 on the host: xb[p, c, e*D + d] = (alpha/2) X[c*256+2p+e, d]
            xb = persist.tile([128, NCH2 * 2 * D], bf16)
            nc.scalar.dma_start(out=xb[:], in_=xb_in[:])
            xv = xb[:].rearrange("p (c d) -> p c d", d=2 * D)

            # xmod[p, b*D+d] = X[b*128+p, d] + (alpha/2) colsum(X)[d]  (host)
            xmods = persist.tile([128, NBATCH * D], fp32)
            nc.scalar.dma_start(
                out=xmods[:].rearrange("p (b d) -> p b d", d=D),
                in_=xm_in.rearrange("(b p) d -> p b d", p=128))

            cnt_all = persist.tile([128, NBATCH], fp32)

            # deferred epilogue: (psum tile, batch index) awaiting store
            pending = []

            def flush_pending():
                if not pending:
                    return
                ps, bp = pending.pop()
                ot = small.tile([128, D], fp32, tag="ot")
                nc.vector.scalar_tensor_tensor(
                    out=ot[:], in0=ps[:], scalar=1.0,
                    in1=xmods[:, bp * D:(bp + 1) * D], op0=mult, op1=add)
                nc.sync.dma_start(
                    out=out_ext[bp * BATCH:(bp + 1) * BATCH, :], in_=ot[:])

            def batch_body(b):
                if b + 2 < NBATCH:
                    load_at(b + 2)
                atile = at_tiles.pop(b)

                # per-segment top-8 candidates
                cands = small.tile([128, nseg * 8], fp16)
                for s in range(nseg):
                    nc.vector.max(out=cands[:, s * 8:(s + 1) * 8],
                                  in_=atile[:, s * seg:(s + 1) * seg])

                # top-32 of candidates -> t32
                v8 = small.tile([128, K], fp16)
                for r in range(4):
                    nc.vector.max(out=v8[:, r * 8:(r + 1) * 8], in_=cands[:])
                    if r < 3:
                        nc.vector.match_replace(
                            out=cands[:], in_to_replace=v8[:, r * 8:(r + 1) * 8],
                            in_values=cands[:], imm_value=NEG_BIG)

                # previous batch's epilogue goes here: by now its matmuls are
                # one full pipeline stage old, so the DVE FIFO never blocks
                # this batch's scan on them.
                flush_pending()

                # ntp = 2^-25 - t32: Sign(y + ntp) is +1 for y >= t32 (incl.
                # the exact tie, fp16 grid gap >= 2^-24 > 2^-25), -1 below.
                ntp = small.tile([128, 1], fp32)
                nc.scalar.activation(out=ntp[:], in_=v8[:, K - 1:K], func=Copy,
                                     scale=-1.0, bias=EPS_TIE)

                # maskpm = Sign(A - t32 + eps) in {+1,-1} fp8, full row in one
                # ACT pass; fused accum -> detector (== 2K - N iff exact)
                maskb = mpool.tile([128, N], fp8, tag="mb")
                nc.scalar.activation(
                    out=maskb[:], in_=atile[:], func=Sign,
                    bias=ntp[:, 0:1], scale=1.0,
                    accum_out=cnt_all[:, b:b + 1])

                # one full-row 16-bit-granularity xbar transpose per batch
                # (SWDGE on Pool: its desc-gen blocks no other engine's ring)
                maskT = mtpool.tile([128, N // 2], u16, tag="mt")
                nc.gpsimd.dma_gather(
                    out_ap=maskT[:].rearrange("p (c i) -> p c i", i=128),
                    in_ap=maskb[:], idxs_ap=tidx[:],
                    num_idxs=128, num_idxs_reg=128, elem_size=N // 2,
                    transpose=True,
                    sbuf_tokens_per_rank=128, sbuf_free_dim_per_rank=N)
                mview = maskT[:].bitcast(fp8).rearrange(
                    "p (c i e) -> p c i e", i=128, e=2)

                ps = psum_pool.tile([128, D], fp32)
                for c in range(NCH2):
                    for e in range(2):
                        nc.tensor.matmul(
                            ps[:], lhsT=mview[:, c, :, e],
                            rhs=xv[:, c, e * D:(e + 1) * D],
                            start=(c == 0 and e == 0),
                            stop=(c == NCH2 - 1 and e == 1))
                pending.append((ps, b))

            if loop_reps == 1:
                for b in range(NBATCH):
                    batch_body(b)
                flush_pending()
            else:
                with tc.For_i(0, loop_reps, 1):
                    load_at(0)
                    load_at(1)
                    for b in range(NBATCH):
                        batch_body(b)
                    flush_pending()

            # counts: cnt_all[p, b] -> count[b*128 + p]
            nc.sync.dma_start(
                out=cnt_ext.rearrange("(b p) one -> p (b one)", p=128),
                in_=cnt_all[:],
            )
    nc.compile()
    return nc


def _tidx():
    t = np.zeros((16, 8), np.int16)
    for i in range(128):
        t[i % 16, i // 16] = i
    return np.tile(t, (8, 1))


def make_in_maps(A, X, alpha):
    import ml_dtypes
    half_a = np.float32(alpha) / np.float32(2.0)
    Xs = (X * half_a).astype(ml_dtypes.bfloat16)
    # xb layout: xb[p, c, e*D + d] = (alpha/2) X[c*256 + 2p + e, d]
    X2 = Xs.reshape(NCH2, 128, 2, D)
    xb = np.ascontiguousarray(np.transpose(X2, (1, 0, 2, 3))).reshape(
        128, NCH2 * 2 * D)
    # xmod = X_self + colsum of the *bf16-rounded scaled* X (so the +1/-1
    # colsum term cancels against what the matmul actually accumulated)
    cs = Xs.astype(np.float64).sum(axis=0)
    xmod = (X.astype(np.float64) + cs[None, :]).astype(np.float32)
    tidx = _tidx()
    # fp16 of (A - SHIFT): monotone, so device top-32 matches fp32 top-32
    # except for fp16 grid collisions at the rank-32 boundary (detected).
    a16 = np.clip(A - np.float32(SHIFT), -60000.0, 60000.0).astype(np.float16)
    return [{
        "a": a16[c * RPC:(c + 1) * RPC],
        "xb": xb,
        "xmod": xmod[c * RPC:(c + 1) * RPC],
        "tidx": tidx,
    } for c in range(NCORES)]


def kernel(**inputs):
    global last_results
    from concourse.bass_utils import run_bass_kernel_spmd

    A = np.ascontiguousarray(np.asarray(inputs["A"], dtype=np.float32))
    X = np.ascontiguousarray(np.asarray(inputs["X"], dtype=np.float32))
    alpha = np.float32(np.asarray(inputs["alpha"]))
    k = int(np.asarray(inputs["k"]))
    assert A.shape == (N, N) and X.shape == (N, D)
    if k != K or alpha == 0.0:
        # Safety net for an unexpected k (or alpha=0): exact host computation.
        idx = np.argsort(-A, axis=1, kind="stable")[:, :k]
        agg = X[idx].sum(axis=1, dtype=np.float32)
        return (X + alpha * agg).astype(np.float32)

    nc = _build_cached()
    in_maps = make_in_maps(A, X, alpha)

    trace = bool(int(os.environ.get("TOPK_TRACE", "0")))
    res = run_bass_kernel_spmd(nc, in_maps, core_ids=list(range(NCORES)),
                               trace=trace)
    last_results = res

    out = np.concatenate([r["out"] for r in res.results], axis=0)
    accs = np.concatenate([r["count"] for r in res.results], axis=0)[:, 0]

    # Host fallback for rows where the device selection is not exactly top-k
    # (fp16 boundary ties, segment overflow, Sign hitting exact zero).
    bad = np.flatnonzero(accs != np.float32(2 * K - N))
    for r in bad:
        order = np.argsort(-A[r], kind="stable")[:K]
        out[r] = X[r] + alpha * X[order].sum(axis=0, dtype=np.float32)

    return out.astype(np.float32, copy=False)
